# revision 60
# baseline (speedup 1.0000x reference)
"""Trainium2 Bass kernel for the CO2-electrolysis surrogate model (v10).

Contract: kernel(**inputs) takes FULL unsharded inputs (x [16384,5], MLP
weights, kinetic params i0/alpha) and returns the FULL [16384,2] output.
Batch is sharded 2048-per-core across 8 NeuronCores (pure data parallel).

Optimized for SINGLE-SHOT latency (the graded metric).  Key structure:
- Per-core shard splits into 2 chunks of 1024 samples that pipeline:
  chunk B's MLP overlaps chunk A's physics tail, and the two tails
  dovetail across engines (B's inputs intentionally DMA-land later so A
  leads and the chains anti-phase).
- Stacked MLP: both 512-sample blocks of a chunk ride the 128
  partitions at once via block-diagonal weights [[W,0],[0,W]] (block
  0's hidden units on partitions 0:64, block 1's on 64:128) — each
  layer is ONE 512-col matmul and ONE full-width relu; PE and relu
  work halves vs a 64-partition layout.
- The voltage search is a 3-step 10-ary climb (grid strides 100/10/1,
  9 probes each) over a virtual 1024-grid starting at b=-1.  State is
  AE[k] = 1/i_kin at the current index; probes multiply by baked factor
  tables exp(sc_k*stride*j).  Step math runs in bf16 (DVE 2x mode; a
  bf16 shadow of AE and of the factor tables feeds the probe ops; C
  rides in a bf16 twin) — worst case this flips the selected grid index
  by ~1 for samples whose crossing sits within bf16 eps of a probe,
  which the 2e-2 tolerance absorbs.  f32 state (AE itself) is kept
  exactly, and the final FE is re-evaluated in f32.
- The final step evaluates probes j=0..10 and fuses the closest-point
  refine: v = #{j in 0..9 : it_j + it_{j+1} > 2*target} directly yields
  the grid index the reference's argmin picks (i_tot is monotone), with
  a clamp v>=1 when b2==-1 (crossing below grid 0).  FE at b2+v is then
  re-evaluated in f32 via one exp.  b itself is never materialized.
- All input DMAs issue at t=0 on the SP queue ordered hdr, xT-A, xT-B;
  dummy matmuls on scratch warm the PE clock ramp during the ~3.5us DMA
  latency window; the activation table set 6 is locked once (exp/ln/
  relu) so no mid-program table reloads occur.
- Host-side prep: x transposed+stacked to xT [10, 1024] (chunk c cols
  [512c, 512c+512), rows b*5+k for block b), |zlt|, factor tables and
  block-diagonal weight pack ride in one fused hdr tensor.
- Output FE pairs DMA out per chunk as soon as each chunk's tail ends.
- Engine placement of each elementwise op was tuned empirically against
  the TimelineSim cost model (DVE does most small chain ops; Pool takes
  off-critical shadows/updates; ACT only exp/ln/relu).
"""

import sys

for _p in ("/opt/trn_rl_repo", "/opt/pypackages"):
    if _p not in sys.path:
        sys.path.insert(0, _p)

import math

import numpy as np

import concourse.bacc as bacc
import concourse.bass as bass
import concourse.tile as tile
from concourse import mybir

F32 = mybir.dt.float32
F32R = mybir.dt.float32r
BF16 = mybir.dt.bfloat16
AF = mybir.ActivationFunctionType
OP = mybir.AluOpType

# ---- problem constants (match reference.py) ----
N = 16384
NCORES = 8
NPC = N // NCORES            # 2048 samples per core
NT = 16                      # 128-sample tiles per core
CH = 2                       # chunks per core
NTC = NT // CH               # tiles per chunk
CPC = NPC // CH              # samples per chunk (1024)
HID = 64
GRID = 1000
VMIN, VMAX = -1.25, 0.0
I_TARGET = 200.0
F_CONST = 96485.33
RT = 8.314 * 298.15
D_CO2 = 1.91e-9
C_CO2 = 34.0
E_EQ = (-0.11, 0.08, 0.0)
N_ELEC_CO2 = (2.0, 12.0)
DV = (VMAX - VMIN) / (GRID - 1)
FRT = F_CONST / RT

# ---- hdr column layout ----
# blob sub-offsets (within BL0):
FB1 = 0            # 27: exp(sc*100*j), j=1..9, k minor
FB2 = 27           # 27: exp(sc*10*j)
FB3 = 54           # 33: exp(sc*j), j=0..10
GS1 = 87           # 3: sc*100
GS2 = 90           # 3: sc*10
GS3 = 93           # 3: sc*1
CAE = 96           # 3: cAE = exp(t0 - sc)/i0
CIL = 99           # 3: [cil0, cil1, 0]
HALF = 102         # 1: 0.5
LN4 = 103          # 1: ln(4e-8)
ZEROC = 104        # 1: 0.0
NB = 105
BL0 = NT                     # blob base (after azlt[16])
B4C = BL0 + NB               # 6: b4
WD1C = B4C + 6               # Wd1 [10, 128]: block-diag W1
WD2C = WD1C + 128            # Wd2 [128, 128]: block-diag W2
WD3C = WD2C + 128            # Wd3 [128, 128]: block-diag W3
W4C = WD3C + 128             # W4b [128, 6]: W4 on both partition halves
BIASC = W4C + 6              # stacked biases [b;b] for b1,b2,b3 (3 cols)
HC = BIASC + 3

_DBG_STAGE = 0


def _make_blob_row(i0, alpha):
    i0 = np.asarray(i0, np.float64)
    alpha = np.asarray(alpha, np.float64)
    sc = [float(alpha[k] * FRT * DV) for k in range(3)]
    t0 = [float(alpha[k] * FRT * (VMIN - E_EQ[k])) for k in range(3)]
    cols = []
    for j in range(1, 10):          # FB1
        for k in range(3):
            cols.append(math.exp(sc[k] * 100 * j))
    for j in range(1, 10):          # FB2
        for k in range(3):
            cols.append(math.exp(sc[k] * 10 * j))
    for j in range(0, 11):          # FB3
        for k in range(3):
            cols.append(math.exp(sc[k] * j))
    for k in range(3):              # GS1
        cols.append(sc[k] * 100.0)
    for k in range(3):              # GS2
        cols.append(sc[k] * 10.0)
    for k in range(3):              # GS3
        cols.append(sc[k])
    for k in range(3):              # CAE
        cols.append(math.exp(t0[k] - sc[k]) / float(i0[k]))
    for nk in N_ELEC_CO2:           # CIL
        cols.append(1.0 / (float(np.float32(np.float32(nk) * np.float32(F_CONST)))
                           * C_CO2 * D_CO2))
    cols.append(0.0)
    cols.append(0.5)                # HALF
    cols.append(math.log(4e-8))     # LN4
    cols.append(0.0)                # ZEROC
    row = np.asarray(cols, np.float32)
    assert row.size == NB, row.size
    return row


class _Pools:
    pass


def _mk_pools(ctx, tc):
    p = _Pools()
    p.io = ctx.enter_context(tc.tile_pool(name="io", bufs=2))
    p.work = ctx.enter_context(tc.tile_pool(name="work", bufs=2))
    p.psum = ctx.enter_context(tc.tile_pool(name="psum", bufs=1, space="PSUM"))
    return p


def _apx(t, off, dims):
    """AP into tile t at free-offset off with explicit (stride, n) dims."""
    return bass.AP(tensor=t.tensor, offset=t.offset + off,
                   ap=[list(t.ap[0])] + [list(d) for d in dims])


def _prologue(tc, po, io, first):
    nc = tc.nc
    xT_d, hdr_d, out_d = io
    hdrsb = po.io.tile([128, HC], F32R, tag="hdr", name="hdr")
    xTsb = []
    if first:
        # lock activation table set 6 (covers exp/ln/relu): zero reloads
        inst = mybir.InstLoadActFuncSet(
            name=nc.get_next_instruction_name(), act_func_set_id=6, ins=[], outs=[])
        nc.scalar.add_instruction(inst)
        # PE p-state warmup: junk matmuls on scratch during the input-DMA
        # latency window (values land in a dead PSUM bank)
        dmy = po.io.tile([2, 576], F32R, tag="dmy", name="dmy", bufs=1)
        dps = po.psum.tile([64, 512], F32, tag="dmy", name="dmyps", bufs=1)
        nc.gpsimd.memset(dmy.bitcast(F32), 1.0)
        for _ in range(5):
            nc.tensor.matmul(dps, dmy[0:2, 0:64], dmy[0:2, 64:576])
    nc.sync.dma_start(hdrsb, hdr_d)
    for c, eng in zip(range(CH), (nc.sync, nc.sync)):
        t = po.io.tile([10, 512], F32R, tag=f"xT{c}", name=f"xT{c}")
        eng.dma_start(t, xT_d[:, c * 512:(c + 1) * 512])
        xTsb.append(t)
    # bf16 shadow of the probe factor tables (FB1|FB2|FB3, 87 cols):
    # all-bf16 operands put the probe-build TTs in DVE's 2x mode
    hdrf = hdrsb.bitcast(F32)
    Fh = po.io.tile([128, 87], BF16, tag="Fh", name="Fh")
    zc = _apx(hdrf, BL0 + ZEROC, [(0, 87)])
    nc.gpsimd.tensor_tensor(Fh, hdrf[:, BL0 + FB1:BL0 + FB1 + 87], zc, OP.add)
    return hdrsb, xTsb, Fh


def _mlp(tc, po, c, hdrsb, xTsb, latout):
    """Generator: 4-layer MLP for chunk c -> latout[0] = lat3 [128, NTC, 6].

    Both 512-sample blocks of the chunk ride the 128 partitions at once:
    block-diagonal weights [[W,0],[0,W]] put block 0's hidden units on
    partitions 0:64 and block 1's on 64:128, so each layer is ONE matmul
    and ONE full-width relu."""
    nc = tc.nc
    hdrf = hdrsb.bitcast(F32)
    Wd1 = bass.AP(tensor=hdrsb.tensor, offset=hdrsb.offset + WD1C,
                  ap=[[list(hdrsb.ap[0])[0], 10], [1, 128]])
    Ws = [Wd1, hdrsb[:, WD2C:WD2C + 128], hdrsb[:, WD3C:WD3C + 128]]
    W4b = hdrsb[:, W4C:W4C + 6]
    biases = [hdrsb[:, BIASC + i:BIASC + i + 1].bitcast(F32)
              for i in range(3)]

    h = None
    for L in range(3):
        src = xTsb if L == 0 else h
        ps = po.psum.tile([128, 512], F32, tag=f"ps{c}", name=f"ps{L}{c}")
        h = po.work.tile([128, 512], F32R, tag=f"h{L}{c}", name=f"h{L}{c}")
        nc.tensor.matmul(ps, Ws[L], src)
        if c == 1:
            nc.scalar.activation(h, ps, AF.Relu, bias=biases[L], scale=1.0)
        else:
            nc.vector.tensor_scalar(h, ps, biases[L], 0.0, OP.add, OP.max)
        yield
    latps = po.psum.tile([128, NTC, 6], F32, tag="lat", name=f"lat{c}", bufs=2)
    for t in range(NTC):
        half, tt = (0, t) if t < NTC // 2 else (64, t - NTC // 2)
        nc.tensor.matmul(latps[:, t, :],
                         h[half:half + 64, tt * 128:(tt + 1) * 128],
                         W4b[half:half + 64, :])
    lat3 = po.work.tile([128, NTC, 6], F32, tag=f"lat3{c}", name=f"lat3{c}")
    b4b = _apx(hdrf, B4C, [(0, NTC), (1, 6)])
    nc.vector.scalar_tensor_tensor(lat3, latps, 1.0, b4b, OP.mult, OP.add)
    latout[0] = lat3
    yield


def _tail(tc, po, c, hdrsb, Fh, latout, out_d):
    """Generator: physics + 3-step climb + fused refine for chunk c."""
    nc = tc.nc
    hdrf = hdrsb.bitcast(F32)
    T = NTC

    def w(name, *dims):
        return po.work.tile([128, *dims], F32, tag=f"{name}{c}",
                            name=f"{name}{c}")

    def bcol(off, dims):
        return _apx(hdrf, BL0 + off, dims)

    azlt = hdrf[:, c * T:(c + 1) * T]
    l3 = latout[0]

    # ---- parameters ----
    a1, a2, a3, d1, a4, e1i, Lt, s5, t6, st = (
        w(n, T) for n in ("a1", "a2", "a3", "d1", "a4", "e1i", "Lt", "s5",
                          "t6", "st"))
    T3, iT, cst, AE = (w(n, T, 3) for n in ("T3", "iT", "cst", "AE"))
    C3 = po.work.tile([128, T, 2], F32, tag=f"C3{c}", name=f"C3{c}")
    C3h = po.work.tile([128, T, 2], BF16, tag=f"C3h{c}", name=f"C3h{c}")

    # C-chain first: it gates step 1's P-add (the longest pole)
    nc.scalar.activation(a1, l3[:, :, 1], AF.Exp, scale=-1.0)        # e^-l1
    nc.gpsimd.tensor_tensor(d1, l3[:, :, 0], l3[:, :, 2], OP.subtract)
    nc.scalar.activation(a2, a1, AF.Ln, bias=1.0, scale=1.0)   # ln(1+e^-l1)
    nc.scalar.activation(a3, a2, AF.Exp, scale=1.5)            # eps^-1.5
    nc.scalar.activation(a4, d1, AF.Exp,
                         bias=hdrf[:, BL0 + LN4:BL0 + LN4 + 1], scale=1.0)
    nc.vector.reciprocal(e1i, a1)                                    # e^l1
    nc.vector.scalar_tensor_tensor(Lt, e1i, 1.0, azlt, OP.add, OP.mult)
    yield
    nc.vector.tensor_tensor(s5, a4, Lt, OP.add)
    nc.gpsimd.tensor_tensor(t6, s5, a3, OP.mult)
    cil = bcol(CIL, [(0, T), (1, 2)])
    t6b = _apx(t6, 0, [(1, T), (0, 2)])
    nc.gpsimd.tensor_tensor(C3, t6b, cil, OP.mult)
    nc.gpsimd.tensor_tensor(C3h, t6b, cil, OP.mult)  # bf16 twin for probes
    # softmax -> AE = 1/i_kin at b=-1 = cAE * st / T3
    nc.scalar.activation(T3, l3[:, :, 3:6], AF.Exp, scale=2.0)
    nc.vector.reduce_sum(st, T3, axis=mybir.AxisListType.X, opt_input=False)
    nc.vector.reciprocal(iT, T3)
    cAEb = bcol(CAE, [(0, T), (1, 3)])
    stb = _apx(st, 0, [(1, T), (0, 3)])
    nc.gpsimd.tensor_tensor(cst, stb, cAEb, OP.mult)
    nc.vector.tensor_tensor(AE, cst, iT, OP.mult)     # 1/i_kin at b=-1
    AEh = po.work.tile([128, T, 3], BF16, tag=f"AEh{c}", name=f"AEh{c}")
    nc.vector.tensor_tensor(AEh, cst, iT, OP.mult)    # bf16 twin, in parallel
    yield

    if _DBG_STAGE == 1:
        o = w("dbg", T, 2)
        nc.vector.tensor_copy(o[:, :, 0], C3[:, :, 0])
        nc.vector.tensor_copy(o[:, :, 1], AE[:, :, 0])
        nc.sync.dma_start(
            out_d.rearrange("(p t) c -> p t c", t=NT)[:, c * T:(c + 1) * T, :], o)
        return

    # ---- climb steps 1, 2 (strides 100, 10; probes j=1..9) ----
    us = []
    AEcur, AEhcur = AE, AEh
    for si, (FB, GS) in enumerate(((FB1, GS1), (FB2, GS2))):
        AEp = po.work.tile([128, T, 9, 3], BF16, tag=f"AEp{si}{c}",
                           name=f"AEp{si}{c}")
        S = po.work.tile([128, T, 9, 3], BF16, tag=f"S{si}{c}",
                         name=f"S{si}{c}")
        it = po.work.tile([128, T, 9], BF16, tag=f"it{si}{c}",
                          name=f"it{si}{c}")
        cp = po.work.tile([128, T, 9], BF16, tag=f"cp{si}{c}",
                          name=f"cp{si}{c}")
        u = w(f"u{si}", T)
        garg = w(f"garg{si}", T, 3)
        G = w(f"G{si}", T, 3)
        AEn = w(f"AEn{si}", T, 3)
        AEb = _apx(AEhcur, 0, [(3, T), (0, 9), (1, 3)])
        Fs = _apx(Fh, FB, [(0, T), (3, 9), (1, 3)])
        nc.vector.tensor_tensor(AEp, AEb, Fs, OP.mult)
        # in-place P-add on k=0,1 only (C_k2 = 0): bf16 all-operands -> 2x DVE
        Chb = _apx(C3h, 0, [(2, T), (0, 9), (1, 2)])
        nc.vector.tensor_tensor(AEp[:, :, :, 0:2], AEp[:, :, :, 0:2],
                                Chb, OP.add)
        with nc.allow_low_precision("probe itot in bf16; refine is f32"):
            nc.vector.reciprocal(S, AEp)
            nc.vector.reduce_sum(it, S, axis=mybir.AxisListType.X,
                                 opt_input=False)
        nc.vector.tensor_scalar(cp, it, I_TARGET, None, OP.is_ge)
        nc.vector.reduce_sum(u, cp, axis=mybir.AxisListType.X, opt_input=False)
        gsb = bcol(GS, [(0, T), (1, 3)])
        ub = _apx(u, 0, [(1, T), (0, 3)])
        nc.gpsimd.tensor_tensor(garg, ub, gsb, OP.mult)
        nc.scalar.activation(G, garg, AF.Exp, scale=1.0)
        nc.vector.tensor_tensor(AEn, AEcur, G, OP.mult)
        AEnh = po.work.tile([128, T, 3], BF16, tag=f"AEnh{si}{c}",
                            name=f"AEnh{si}{c}")
        nc.gpsimd.tensor_tensor(AEnh, AEcur, G, OP.mult)
        us.append(u)
        AEcur, AEhcur = AEn, AEnh
        yield

    if _DBG_STAGE == 2:
        o = w("dbg", T, 2)
        nc.vector.tensor_copy(o[:, :, 0], us[0])
        nc.vector.tensor_copy(o[:, :, 1], us[1])
        nc.sync.dma_start(
            out_d.rearrange("(p t) c -> p t c", t=NT)[:, c * T:(c + 1) * T, :], o)
        return

    # ---- step 3 (stride 1; probes j=0..10) + fused refine ----
    AEp3 = po.work.tile([128, T, 11, 3], BF16, tag=f"AEp3{c}", name=f"AEp3{c}")
    S3 = po.work.tile([128, T, 11, 3], BF16, tag=f"S3{c}", name=f"S3{c}")
    it3 = po.work.tile([128, T, 11], BF16, tag=f"it3{c}", name=f"it3{c}")
    hs = po.work.tile([128, T, 10], BF16, tag=f"hs{c}", name=f"hs{c}")
    cpv = po.work.tile([128, T, 10], BF16, tag=f"cpv{c}", name=f"cpv{c}")
    v = w("v", T)
    usum = w("usum", T)
    nf = w("nf", T)
    v2 = w("v2", T)
    g3 = w("g3", T, 3)
    E3 = w("E3", T, 3)
    AEv = w("AEv", T, 3)
    Sv = w("Sv", T, 3)
    tot = w("tot", T)
    rtot = w("rtot", T)
    fe3 = w("fe3", T, 2)

    AEb = _apx(AEhcur, 0, [(3, T), (0, 11), (1, 3)])
    F3 = _apx(Fh, FB3, [(0, T), (3, 11), (1, 3)])
    nc.vector.tensor_tensor(AEp3, AEb, F3, OP.mult)
    Chb = _apx(C3h, 0, [(2, T), (0, 11), (1, 2)])
    nc.vector.tensor_tensor(AEp3[:, :, :, 0:2], AEp3[:, :, :, 0:2],
                            Chb, OP.add)
    # off-critical: nf = 1 if (u1 + u2) == 0  (i.e. b2 == -1)
    nc.gpsimd.tensor_tensor(usum, us[0], us[1], OP.add)
    nc.vector.tensor_scalar(nf, usum, 0.5, None, OP.is_le)
    with nc.allow_low_precision("probe itot in bf16; refine is f32"):
        nc.vector.reciprocal(S3, AEp3)
        nc.vector.reduce_sum(it3, S3, axis=mybir.AxisListType.X,
                             opt_input=False)
    yield
    with nc.allow_low_precision("probe itot in bf16; refine is f32"):
        nc.vector.tensor_tensor(hs, it3[:, :, 0:10], it3[:, :, 1:11], OP.add)
    nc.vector.tensor_scalar(cpv, hs, 2.0 * I_TARGET, None, OP.is_gt)
    nc.vector.reduce_sum(v, cpv, axis=mybir.AxisListType.X, opt_input=False)
    nc.vector.tensor_tensor(v2, v, nf, OP.max)
    yield
    # FE at the selected grid point b2+v2, re-evaluated directly in f32
    gs3 = bcol(GS3, [(0, T), (1, 3)])
    v2b = _apx(v2, 0, [(1, T), (0, 3)])
    nc.vector.tensor_tensor(g3, v2b, gs3, OP.mult)
    nc.scalar.activation(E3, g3, AF.Exp, scale=1.0)
    nc.gpsimd.tensor_tensor(AEv, AEcur, E3, OP.mult)
    nc.gpsimd.tensor_tensor(AEv[:, :, 0:2], AEv[:, :, 0:2], C3, OP.add)
    nc.vector.reciprocal(Sv, AEv)
    nc.vector.reduce_sum(tot, Sv, axis=mybir.AxisListType.X, opt_input=False)
    nc.vector.reciprocal(rtot, tot)
    rtb = _apx(rtot, 0, [(1, T), (0, 1)])
    nc.vector.tensor_tensor(fe3[:, :, 0:1], Sv[:, :, 1:2], rtb, OP.mult)
    nc.vector.tensor_tensor(fe3[:, :, 1:2], Sv[:, :, 0:1], rtb, OP.mult)
    nc.sync.dma_start(
        out_d.rearrange("(p t) c -> p t c", t=NT)[:, c * T:(c + 1) * T, :], fe3)
    yield


def _drive(items):
    """items: list of (generator, start_round). Round-robin advance."""
    rnd = 0
    items = [(g, s) for g, s in items]
    done = [False] * len(items)
    while not all(done):
        for i, (g, s) in enumerate(items):
            if done[i] or rnd < s:
                continue
            try:
                next(g)
            except StopIteration:
                done[i] = True
        rnd += 1


def _build(reps=1):
    from contextlib import ExitStack

    nc = bacc.Bacc("TRN2", target_bir_lowering=False, debug=False)
    xT_d = nc.dram_tensor("xT", [10, NPC // 2], F32R, kind="ExternalInput").ap()
    hdr_d = nc.dram_tensor("hdr", [128, HC], F32R, kind="ExternalInput").ap()
    out_d = nc.dram_tensor("out", [NPC, 2], F32, kind="ExternalOutput").ap()
    io = (xT_d, hdr_d, out_d)

    with tile.TileContext(nc) as tc:
        with ExitStack() as ctx:
            po = _mk_pools(ctx, tc)
            for r in range(reps):
                hdrsb, xTsb, Fh = _prologue(tc, po, io, first=(r == 0))
                lats = [[None], [None]]
                _drive([(_mlp(tc, po, 0, hdrsb, xTsb[0], lats[0]), 0)])
                _drive([(_mlp(tc, po, 1, hdrsb, xTsb[1], lats[1]), 0),
                        (_tail(tc, po, 0, hdrsb, Fh, lats[0], out_d), 0),
                        (_tail(tc, po, 1, hdrsb, Fh, lats[1], out_d), 4)])
    nc.compile()
    return nc


_CACHE = {}


def _make_inputs(x, W1, b1, W2, b2, W3, b3, W4, b4, i0, alpha):
    x = np.ascontiguousarray(np.asarray(x, np.float32))
    blob_row = _make_blob_row(i0, alpha)
    in_maps = []
    for c in range(NCORES):
        shard = x[c * NPC:(c + 1) * NPC]
        xT5 = shard.reshape(128, NT, 5).transpose(2, 1, 0).reshape(5, NPC)
        # stack each chunk's two 512-sample blocks on the partition axis
        xT = np.ascontiguousarray(
            xT5.reshape(5, CH, 2, 512).transpose(2, 0, 1, 3).reshape(10, NPC // 2))
        hdr = np.zeros((128, HC), np.float32)
        hdr[:, 0:NT] = np.abs(shard[:, 3].reshape(128, NT))
        hdr[:, BL0:BL0 + NB] = blob_row
        hdr[:, B4C:B4C + 6] = np.asarray(b4, np.float32)
        hdr[0:5, WD1C:WD1C + 64] = np.asarray(W1, np.float32)
        hdr[5:10, WD1C + 64:WD1C + 128] = np.asarray(W1, np.float32)
        hdr[0:64, WD2C:WD2C + 64] = np.asarray(W2, np.float32)
        hdr[64:128, WD2C + 64:WD2C + 128] = np.asarray(W2, np.float32)
        hdr[0:64, WD3C:WD3C + 64] = np.asarray(W3, np.float32)
        hdr[64:128, WD3C + 64:WD3C + 128] = np.asarray(W3, np.float32)
        hdr[0:64, W4C:W4C + 6] = np.asarray(W4, np.float32)
        hdr[64:128, W4C:W4C + 6] = np.asarray(W4, np.float32)
        for i, b in enumerate((b1, b2, b3)):
            hdr[0:64, BIASC + i] = np.asarray(b, np.float32)
            hdr[64:128, BIASC + i] = np.asarray(b, np.float32)
        in_maps.append({"xT": xT, "hdr": hdr})
    return in_maps


def kernel(x, W1, b1, W2, b2, W3, b3, W4, b4, i0, alpha):
    from concourse.bass_utils import run_bass_kernel_spmd

    if "nc" not in _CACHE:
        _CACHE["nc"] = _build()
    nc = _CACHE["nc"]
    in_maps = _make_inputs(x, W1, b1, W2, b2, W3, b3, W4, b4, i0, alpha)
    res = run_bass_kernel_spmd(nc, in_maps, core_ids=list(range(NCORES)))
    return np.concatenate([res.results[c]["out"] for c in range(NCORES)], axis=0)


# revision 63
# speedup vs baseline: 1.1418x; 1.1418x over previous
"""Trainium2 Bass kernel for the CO2-electrolysis surrogate model (v10).

Contract: kernel(**inputs) takes FULL unsharded inputs (x [16384,5], MLP
weights, kinetic params i0/alpha) and returns the FULL [16384,2] output.
Batch is sharded 2048-per-core across 8 NeuronCores (pure data parallel).

Optimized for SINGLE-SHOT latency (the graded metric).  Key structure:
- Per-core shard splits into 2 chunks of 1024 samples that pipeline:
  chunk B's MLP overlaps chunk A's physics tail, and the two tails
  dovetail across engines (B's inputs intentionally DMA-land later so A
  leads and the chains anti-phase).
- Stacked MLP: both 512-sample blocks of a chunk ride the 128
  partitions at once via block-diagonal weights [[W,0],[0,W]] (block
  0's hidden units on partitions 0:64, block 1's on 64:128) — each
  layer is ONE 512-col matmul and ONE full-width relu; PE and relu
  work halves vs a 64-partition layout.
- The voltage search is a 3-step 10-ary climb (grid strides 100/10/1,
  9 probes each) over a virtual 1024-grid starting at b=-1.  State is
  AE[k] = 1/i_kin at the current index; probes multiply by baked factor
  tables exp(sc_k*stride*j).  Step math runs in bf16 (DVE 2x mode; a
  bf16 shadow of AE and of the factor tables feeds the probe ops; C
  rides in a bf16 twin) — worst case this flips the selected grid index
  by ~1 for samples whose crossing sits within bf16 eps of a probe,
  which the 2e-2 tolerance absorbs.  f32 state (AE itself) is kept
  exactly, and the final FE is re-evaluated in f32.
- The final step evaluates probes j=0..10 and fuses the closest-point
  refine: v = #{j in 0..9 : it_j + it_{j+1} > 2*target} directly yields
  the grid index the reference's argmin picks (i_tot is monotone), with
  a clamp v>=1 when b2==-1 (crossing below grid 0).  FE at b2+v is then
  re-evaluated in f32 via one exp.  b itself is never materialized.
- All input DMAs issue at t=0 on the SP queue ordered hdr, xT-A, xT-B;
  dummy matmuls on scratch warm the PE clock ramp during the ~3.5us DMA
  latency window; the activation table set 6 is locked once (exp/ln/
  relu) so no mid-program table reloads occur.
- Host-side prep: x transposed+stacked to xT [10, 1024] (chunk c cols
  [512c, 512c+512), rows b*5+k for block b), |zlt|, factor tables and
  block-diagonal weight pack ride in one fused hdr tensor.
- Output FE pairs DMA out per chunk as soon as each chunk's tail ends.
- Engine placement of each elementwise op was tuned empirically against
  the TimelineSim cost model (DVE does most small chain ops; Pool takes
  off-critical shadows/updates; ACT only exp/ln/relu).
"""

import sys

for _p in ("/opt/trn_rl_repo", "/opt/pypackages"):
    if _p not in sys.path:
        sys.path.insert(0, _p)

import math

import numpy as np

import concourse.bacc as bacc
import concourse.bass as bass
import concourse.tile as tile
from concourse import mybir

F32 = mybir.dt.float32
F32R = mybir.dt.float32r
BF16 = mybir.dt.bfloat16
AF = mybir.ActivationFunctionType
OP = mybir.AluOpType

# ---- problem constants (match reference.py) ----
N = 16384
NCORES = 8
NPC = N // NCORES            # 2048 samples per core
NT = 16                      # 128-sample tiles per core
CH = 2                       # chunks per core
NTC = NT // CH               # tiles per chunk
CPC = NPC // CH              # samples per chunk (1024)
HID = 64
GRID = 1000
VMIN, VMAX = -1.25, 0.0
I_TARGET = 200.0
F_CONST = 96485.33
RT = 8.314 * 298.15
D_CO2 = 1.91e-9
C_CO2 = 34.0
E_EQ = (-0.11, 0.08, 0.0)
N_ELEC_CO2 = (2.0, 12.0)
DV = (VMAX - VMIN) / (GRID - 1)
FRT = F_CONST / RT

# ---- hdr column layout ----
# blob sub-offsets (within BL0):
FB1 = 0            # 27: exp(sc*100*j), j=1..9, k minor
FB2 = 27           # 27: exp(sc*10*j)
FB3 = 54           # 33: exp(sc*j), j=0..10
GS1 = 87           # 3: sc*100
GS2 = 90           # 3: sc*10
GS3 = 93           # 3: sc*1
CAE = 96           # 3: cAE = exp(t0 - sc)/i0
CIL = 99           # 3: [cil0, cil1, 0]
HALF = 102         # 1: 0.5
LN4 = 103          # 1: ln(4e-8)
ZEROC = 104        # 1: 0.0
NB = 105
BL0 = NT                     # blob base (after azlt[16])
B4C = BL0 + NB               # 6: b4
WD1C = B4C + 6               # Wd1 [10, 128]: block-diag W1
WD2C = WD1C + 128            # Wd2 [128, 128]: block-diag W2
WD3C = WD2C + 128            # Wd3 [128, 128]: block-diag W3
W4C = WD3C + 128             # W4b [128, 6]: W4 on both partition halves
BIASC = W4C + 6              # stacked biases [b;b] for b1,b2,b3 (3 cols)
HC = BIASC + 3

_DBG_STAGE = 0


def _make_blob_row(i0, alpha):
    i0 = np.asarray(i0, np.float64)
    alpha = np.asarray(alpha, np.float64)
    sc = [float(alpha[k] * FRT * DV) for k in range(3)]
    t0 = [float(alpha[k] * FRT * (VMIN - E_EQ[k])) for k in range(3)]
    cols = []
    for j in range(1, 10):          # FB1
        for k in range(3):
            cols.append(math.exp(sc[k] * 100 * j))
    for j in range(1, 10):          # FB2
        for k in range(3):
            cols.append(math.exp(sc[k] * 10 * j))
    for j in range(0, 11):          # FB3
        for k in range(3):
            cols.append(math.exp(sc[k] * j))
    for k in range(3):              # GS1
        cols.append(sc[k] * 100.0)
    for k in range(3):              # GS2
        cols.append(sc[k] * 10.0)
    for k in range(3):              # GS3
        cols.append(sc[k])
    for k in range(3):              # CAE
        cols.append(math.exp(t0[k] - sc[k]) / float(i0[k]))
    for nk in N_ELEC_CO2:           # CIL
        cols.append(1.0 / (float(np.float32(np.float32(nk) * np.float32(F_CONST)))
                           * C_CO2 * D_CO2))
    cols.append(0.0)
    cols.append(0.5)                # HALF
    cols.append(math.log(4e-8))     # LN4
    cols.append(0.0)                # ZEROC
    row = np.asarray(cols, np.float32)
    assert row.size == NB, row.size
    return row


class _Pools:
    pass


def _mk_pools(ctx, tc):
    p = _Pools()
    p.io = ctx.enter_context(tc.tile_pool(name="io", bufs=2))
    p.work = ctx.enter_context(tc.tile_pool(name="work", bufs=2))
    p.psum = ctx.enter_context(tc.tile_pool(name="psum", bufs=1, space="PSUM"))
    return p


def _apx(t, off, dims):
    """AP into tile t at free-offset off with explicit (stride, n) dims."""
    return bass.AP(tensor=t.tensor, offset=t.offset + off,
                   ap=[list(t.ap[0])] + [list(d) for d in dims])


def _prologue(tc, po, io, first):
    nc = tc.nc
    xT_d, hdr_d, out_d = io
    hdrsb = po.io.tile([128, HC], F32R, tag="hdr", name="hdr")
    xTsb = []
    if first:
        # lock activation table set 6 (covers exp/ln/relu): zero reloads
        inst = mybir.InstLoadActFuncSet(
            name=nc.get_next_instruction_name(), act_func_set_id=6, ins=[], outs=[])
        nc.scalar.add_instruction(inst)
        # PE p-state warmup: junk matmuls on scratch during the input-DMA
        # latency window (values land in a dead PSUM bank)
        dmy = po.io.tile([2, 576], F32R, tag="dmy", name="dmy", bufs=1)
        dps = po.psum.tile([64, 512], F32, tag="dmy", name="dmyps", bufs=1)
        nc.gpsimd.memset(dmy.bitcast(F32), 1.0)
        for _ in range(5):
            nc.tensor.matmul(dps, dmy[0:2, 0:64], dmy[0:2, 64:576])
    nc.sync.dma_start(hdrsb, hdr_d)
    for c, eng in zip(range(CH), (nc.sync, nc.sync)):
        t = po.io.tile([10, 512], F32R, tag=f"xT{c}", name=f"xT{c}")
        eng.dma_start(t, xT_d[:, c * 512:(c + 1) * 512])
        xTsb.append(t)
    # bf16 shadow of the probe factor tables (FB1|FB2|FB3, 87 cols):
    # all-bf16 operands put the probe-build TTs in DVE's 2x mode
    hdrf = hdrsb.bitcast(F32)
    Fh = po.io.tile([128, 87], BF16, tag="Fh", name="Fh")
    zc = _apx(hdrf, BL0 + ZEROC, [(0, 87)])
    nc.gpsimd.tensor_tensor(Fh, hdrf[:, BL0 + FB1:BL0 + FB1 + 87], zc, OP.add)
    return hdrsb, xTsb, Fh


def _mlp(tc, po, c, hdrsb, xTsb, latout):
    """Generator: 4-layer MLP for chunk c -> latout[0] = lat3 [128, NTC, 6].

    Both 512-sample blocks of the chunk ride the 128 partitions at once:
    block-diagonal weights [[W,0],[0,W]] put block 0's hidden units on
    partitions 0:64 and block 1's on 64:128, so each layer is ONE matmul
    and ONE full-width relu."""
    nc = tc.nc
    hdrf = hdrsb.bitcast(F32)
    Wd1 = bass.AP(tensor=hdrsb.tensor, offset=hdrsb.offset + WD1C,
                  ap=[[list(hdrsb.ap[0])[0], 10], [1, 128]])
    Ws = [Wd1, hdrsb[:, WD2C:WD2C + 128], hdrsb[:, WD3C:WD3C + 128]]
    W4b = hdrsb[:, W4C:W4C + 6]
    biases = [hdrsb[:, BIASC + i:BIASC + i + 1].bitcast(F32)
              for i in range(3)]

    h = None
    for L in range(3):
        src = xTsb if L == 0 else h
        ps = po.psum.tile([128, 512], F32, tag=f"ps{c}", name=f"ps{L}{c}")
        h = po.work.tile([128, 512], F32R, tag=f"h{L}{c}", name=f"h{L}{c}")
        nc.tensor.matmul(ps, Ws[L], src)
        if c == 1:
            nc.scalar.activation(h, ps, AF.Relu, bias=biases[L], scale=1.0)
        else:
            nc.vector.tensor_scalar(h, ps, biases[L], 0.0, OP.add, OP.max)
        yield
    latps = po.psum.tile([128, NTC, 6], F32, tag="lat", name=f"lat{c}", bufs=2)
    for t in range(NTC):
        half, tt = (0, t) if t < NTC // 2 else (64, t - NTC // 2)
        nc.tensor.matmul(latps[:, t, :],
                         h[half:half + 64, tt * 128:(tt + 1) * 128],
                         W4b[half:half + 64, :])
    lat3 = po.work.tile([128, NTC, 6], F32, tag=f"lat3{c}", name=f"lat3{c}")
    b4b = _apx(hdrf, B4C, [(0, NTC), (1, 6)])
    nc.vector.scalar_tensor_tensor(lat3, latps, 1.0, b4b, OP.mult, OP.add)
    latout[0] = lat3
    yield


def _tail(tc, po, c, hdrsb, Fh, latout, out_d):
    """Generator: physics + 3-step climb + fused refine for chunk c."""
    nc = tc.nc
    hdrf = hdrsb.bitcast(F32)
    T = NTC

    def w(name, *dims):
        return po.work.tile([128, *dims], F32, tag=f"{name}{c}",
                            name=f"{name}{c}")

    def bcol(off, dims):
        return _apx(hdrf, BL0 + off, dims)

    azlt = hdrf[:, c * T:(c + 1) * T]
    l3 = latout[0]

    # ---- parameters ----
    a1, a2, a3, d1, a4, e1i, Lt, s5, t6, st = (
        w(n, T) for n in ("a1", "a2", "a3", "d1", "a4", "e1i", "Lt", "s5",
                          "t6", "st"))
    T3, iT, cst, AE = (w(n, T, 3) for n in ("T3", "iT", "cst", "AE"))
    C3 = po.work.tile([128, T, 2], F32, tag=f"C3{c}", name=f"C3{c}")
    C3h = po.work.tile([128, T, 2], BF16, tag=f"C3h{c}", name=f"C3h{c}")

    # C-chain first: it gates step 1's P-add (the longest pole)
    nc.scalar.activation(a1, l3[:, :, 1], AF.Exp, scale=-1.0)        # e^-l1
    nc.gpsimd.tensor_tensor(d1, l3[:, :, 0], l3[:, :, 2], OP.subtract)
    nc.scalar.activation(a2, a1, AF.Ln, bias=1.0, scale=1.0)   # ln(1+e^-l1)
    nc.scalar.activation(a3, a2, AF.Exp, scale=1.5)            # eps^-1.5
    nc.scalar.activation(a4, d1, AF.Exp,
                         bias=hdrf[:, BL0 + LN4:BL0 + LN4 + 1], scale=1.0)
    nc.vector.reciprocal(e1i, a1)                                    # e^l1
    nc.vector.scalar_tensor_tensor(Lt, e1i, 1.0, azlt, OP.add, OP.mult)
    yield
    nc.vector.tensor_tensor(s5, a4, Lt, OP.add)
    nc.gpsimd.tensor_tensor(t6, s5, a3, OP.mult)
    cil = bcol(CIL, [(0, T), (1, 2)])
    t6b = _apx(t6, 0, [(1, T), (0, 2)])
    nc.gpsimd.tensor_tensor(C3, t6b, cil, OP.mult)
    nc.gpsimd.tensor_tensor(C3h, t6b, cil, OP.mult)  # bf16 twin for probes
    # softmax -> AE = 1/i_kin at b=-1 = cAE * st / T3
    nc.scalar.activation(T3, l3[:, :, 3:6], AF.Exp, scale=2.0)
    nc.vector.reduce_sum(st, T3, axis=mybir.AxisListType.X, opt_input=False)
    nc.vector.reciprocal(iT, T3)
    cAEb = bcol(CAE, [(0, T), (1, 3)])
    stb = _apx(st, 0, [(1, T), (0, 3)])
    nc.gpsimd.tensor_tensor(cst, stb, cAEb, OP.mult)
    nc.vector.tensor_tensor(AE, cst, iT, OP.mult)     # 1/i_kin at b=-1
    AEh = po.work.tile([128, T, 3], BF16, tag=f"AEh{c}", name=f"AEh{c}")
    nc.vector.tensor_tensor(AEh, cst, iT, OP.mult)    # bf16 twin, in parallel
    yield

    if _DBG_STAGE == 1:
        o = w("dbg", T, 2)
        nc.vector.tensor_copy(o[:, :, 0], C3[:, :, 0])
        nc.vector.tensor_copy(o[:, :, 1], AE[:, :, 0])
        nc.sync.dma_start(
            out_d.rearrange("(p t) c -> p t c", t=NT)[:, c * T:(c + 1) * T, :], o)
        return

    # ---- climb steps 1, 2 (strides 100, 10; probes j=1..9) ----
    us = []
    AEcur, AEhcur = AE, AEh
    for si, (FB, GS) in enumerate(((FB1, GS1), (FB2, GS2))):
        AEp = po.work.tile([128, T, 9, 3], BF16, tag=f"AEp{si}{c}",
                           name=f"AEp{si}{c}")
        S = po.work.tile([128, T, 9, 3], BF16, tag=f"S{si}{c}",
                         name=f"S{si}{c}")
        it = po.work.tile([128, T, 9], BF16, tag=f"it{si}{c}",
                          name=f"it{si}{c}")
        cp = po.work.tile([128, T, 9], BF16, tag=f"cp{si}{c}",
                          name=f"cp{si}{c}")
        u = w(f"u{si}", T)
        garg = w(f"garg{si}", T, 3)
        G = w(f"G{si}", T, 3)
        AEn = w(f"AEn{si}", T, 3)
        AEb = _apx(AEhcur, 0, [(3, T), (0, 9), (1, 3)])
        Fs = _apx(Fh, FB, [(0, T), (3, 9), (1, 3)])
        nc.vector.tensor_tensor(AEp, AEb, Fs, OP.mult)
        # in-place P-add on k=0,1 only (C_k2 = 0): bf16 all-operands -> 2x DVE
        Chb = _apx(C3h, 0, [(2, T), (0, 9), (1, 2)])
        nc.vector.tensor_tensor(AEp[:, :, :, 0:2], AEp[:, :, :, 0:2],
                                Chb, OP.add)
        with nc.allow_low_precision("probe itot in bf16; refine is f32"):
            nc.vector.reciprocal(S, AEp)
            nc.vector.reduce_sum(it, S, axis=mybir.AxisListType.X,
                                 opt_input=False)
        nc.vector.tensor_scalar(cp, it, I_TARGET, None, OP.is_ge)
        nc.vector.reduce_sum(u, cp, axis=mybir.AxisListType.X, opt_input=False)
        gsb = bcol(GS, [(0, T), (1, 3)])
        ub = _apx(u, 0, [(1, T), (0, 3)])
        nc.gpsimd.tensor_tensor(garg, ub, gsb, OP.mult)
        nc.scalar.activation(G, garg, AF.Exp, scale=1.0)
        nc.vector.tensor_tensor(AEn, AEcur, G, OP.mult)
        AEnh = po.work.tile([128, T, 3], BF16, tag=f"AEnh{si}{c}",
                            name=f"AEnh{si}{c}")
        nc.gpsimd.tensor_tensor(AEnh, AEcur, G, OP.mult)
        us.append(u)
        AEcur, AEhcur = AEn, AEnh
        yield

    if _DBG_STAGE == 2:
        o = w("dbg", T, 2)
        nc.vector.tensor_copy(o[:, :, 0], us[0])
        nc.vector.tensor_copy(o[:, :, 1], us[1])
        nc.sync.dma_start(
            out_d.rearrange("(p t) c -> p t c", t=NT)[:, c * T:(c + 1) * T, :], o)
        return

    # ---- step 3 (stride 1; probes j=0..10) + fused refine ----
    AEp3 = po.work.tile([128, T, 11, 3], BF16, tag=f"AEp3{c}", name=f"AEp3{c}")
    S3 = po.work.tile([128, T, 11, 3], BF16, tag=f"S3{c}", name=f"S3{c}")
    it3 = po.work.tile([128, T, 11], BF16, tag=f"it3{c}", name=f"it3{c}")
    hs = po.work.tile([128, T, 10], BF16, tag=f"hs{c}", name=f"hs{c}")
    cpv = po.work.tile([128, T, 10], BF16, tag=f"cpv{c}", name=f"cpv{c}")
    v = w("v", T)
    usum = w("usum", T)
    nf = w("nf", T)
    v2 = w("v2", T)
    g3 = w("g3", T, 3)
    E3 = w("E3", T, 3)
    AEv = w("AEv", T, 3)
    Sv = w("Sv", T, 3)
    tot = w("tot", T)
    rtot = w("rtot", T)
    fe3 = w("fe3", T, 2)

    AEb = _apx(AEhcur, 0, [(3, T), (0, 11), (1, 3)])
    F3 = _apx(Fh, FB3, [(0, T), (3, 11), (1, 3)])
    nc.vector.tensor_tensor(AEp3, AEb, F3, OP.mult)
    Chb = _apx(C3h, 0, [(2, T), (0, 11), (1, 2)])
    nc.vector.tensor_tensor(AEp3[:, :, :, 0:2], AEp3[:, :, :, 0:2],
                            Chb, OP.add)
    # off-critical: nf = 1 if (u1 + u2) == 0  (i.e. b2 == -1)
    nc.gpsimd.tensor_tensor(usum, us[0], us[1], OP.add)
    nc.vector.tensor_scalar(nf, usum, 0.5, None, OP.is_le)
    with nc.allow_low_precision("probe itot in bf16; refine is f32"):
        nc.vector.reciprocal(S3, AEp3)
        nc.vector.reduce_sum(it3, S3, axis=mybir.AxisListType.X,
                             opt_input=False)
    yield
    with nc.allow_low_precision("probe itot in bf16; refine is f32"):
        nc.vector.tensor_tensor(hs, it3[:, :, 0:10], it3[:, :, 1:11], OP.add)
    nc.vector.tensor_scalar(cpv, hs, 2.0 * I_TARGET, None, OP.is_gt)
    nc.vector.reduce_sum(v, cpv, axis=mybir.AxisListType.X, opt_input=False)
    nc.vector.tensor_tensor(v2, v, nf, OP.max)
    yield
    # FE at the selected grid point b2+v2, re-evaluated directly in f32
    gs3 = bcol(GS3, [(0, T), (1, 3)])
    v2b = _apx(v2, 0, [(1, T), (0, 3)])
    nc.vector.tensor_tensor(g3, v2b, gs3, OP.mult)
    nc.scalar.activation(E3, g3, AF.Exp, scale=1.0)
    nc.gpsimd.tensor_tensor(AEv, AEcur, E3, OP.mult)
    nc.gpsimd.tensor_tensor(AEv[:, :, 0:2], AEv[:, :, 0:2], C3, OP.add)
    nc.vector.reciprocal(Sv, AEv)
    nc.vector.reduce_sum(tot, Sv, axis=mybir.AxisListType.X, opt_input=False)
    nc.vector.reciprocal(rtot, tot)
    rtb = _apx(rtot, 0, [(1, T), (0, 1)])
    Svr = _apx(Sv, 1, [(3, T), (-1, 2)])    # k reversed: [S1, S0]
    rtb2 = _apx(rtot, 0, [(1, T), (0, 2)])
    nc.vector.tensor_tensor(fe3, Svr, rtb2, OP.mult)
    nc.sync.dma_start(
        out_d.rearrange("(p t) c -> p t c", t=NT)[:, c * T:(c + 1) * T, :], fe3)
    yield


def _drive(items):
    """items: list of (generator, start_round). Round-robin advance."""
    rnd = 0
    items = [(g, s) for g, s in items]
    done = [False] * len(items)
    while not all(done):
        for i, (g, s) in enumerate(items):
            if done[i] or rnd < s:
                continue
            try:
                next(g)
            except StopIteration:
                done[i] = True
        rnd += 1


def _build(reps=1):
    from contextlib import ExitStack

    nc = bacc.Bacc("TRN2", target_bir_lowering=False, debug=False)
    xT_d = nc.dram_tensor("xT", [10, NPC // 2], F32R, kind="ExternalInput").ap()
    hdr_d = nc.dram_tensor("hdr", [128, HC], F32R, kind="ExternalInput").ap()
    out_d = nc.dram_tensor("out", [NPC, 2], F32, kind="ExternalOutput").ap()
    io = (xT_d, hdr_d, out_d)

    with tile.TileContext(nc) as tc:
        with ExitStack() as ctx:
            po = _mk_pools(ctx, tc)
            for r in range(reps):
                hdrsb, xTsb, Fh = _prologue(tc, po, io, first=(r == 0))
                lats = [[None], [None]]
                _drive([(_mlp(tc, po, 0, hdrsb, xTsb[0], lats[0]), 0)])
                _drive([(_mlp(tc, po, 1, hdrsb, xTsb[1], lats[1]), 0),
                        (_tail(tc, po, 0, hdrsb, Fh, lats[0], out_d), 0),
                        (_tail(tc, po, 1, hdrsb, Fh, lats[1], out_d), 4)])
    nc.compile()
    return nc


_CACHE = {}


def _make_inputs(x, W1, b1, W2, b2, W3, b3, W4, b4, i0, alpha):
    x = np.ascontiguousarray(np.asarray(x, np.float32))
    blob_row = _make_blob_row(i0, alpha)
    in_maps = []
    for c in range(NCORES):
        shard = x[c * NPC:(c + 1) * NPC]
        xT5 = shard.reshape(128, NT, 5).transpose(2, 1, 0).reshape(5, NPC)
        # stack each chunk's two 512-sample blocks on the partition axis
        xT = np.ascontiguousarray(
            xT5.reshape(5, CH, 2, 512).transpose(2, 0, 1, 3).reshape(10, NPC // 2))
        hdr = np.zeros((128, HC), np.float32)
        hdr[:, 0:NT] = np.abs(shard[:, 3].reshape(128, NT))
        hdr[:, BL0:BL0 + NB] = blob_row
        hdr[:, B4C:B4C + 6] = np.asarray(b4, np.float32)
        hdr[0:5, WD1C:WD1C + 64] = np.asarray(W1, np.float32)
        hdr[5:10, WD1C + 64:WD1C + 128] = np.asarray(W1, np.float32)
        hdr[0:64, WD2C:WD2C + 64] = np.asarray(W2, np.float32)
        hdr[64:128, WD2C + 64:WD2C + 128] = np.asarray(W2, np.float32)
        hdr[0:64, WD3C:WD3C + 64] = np.asarray(W3, np.float32)
        hdr[64:128, WD3C + 64:WD3C + 128] = np.asarray(W3, np.float32)
        hdr[0:64, W4C:W4C + 6] = np.asarray(W4, np.float32)
        hdr[64:128, W4C:W4C + 6] = np.asarray(W4, np.float32)
        for i, b in enumerate((b1, b2, b3)):
            hdr[0:64, BIASC + i] = np.asarray(b, np.float32)
            hdr[64:128, BIASC + i] = np.asarray(b, np.float32)
        in_maps.append({"xT": xT, "hdr": hdr})
    return in_maps


def kernel(x, W1, b1, W2, b2, W3, b3, W4, b4, i0, alpha):
    from concourse.bass_utils import run_bass_kernel_spmd

    if "nc" not in _CACHE:
        _CACHE["nc"] = _build()
    nc = _CACHE["nc"]
    in_maps = _make_inputs(x, W1, b1, W2, b2, W3, b3, W4, b4, i0, alpha)
    res = run_bass_kernel_spmd(nc, in_maps, core_ids=list(range(NCORES)))
    return np.concatenate([res.results[c]["out"] for c in range(NCORES)], axis=0)
